# revision 42
# baseline (speedup 1.0000x reference)
"""Trainium2 Bass kernel for a 2-layer GRU (B=4096, T=128, D=32, H=64) + linear head.

Strategy
--------
Data-parallel over batch: B=4096 -> 8 NeuronCores x 512. Each core runs the
full T=128 recurrence for its batch shard, gate-major on chip: gates/hidden
on partitions, batch on the free dim.

The two GRU layers run as a wavefront (layer 1 one step behind layer 0) and
are FUSED onto shared partition ranges: layer 0 owns partitions 0:64, layer 1
owns 64:128 of four cross-layer PSUM gate banks ([z0|z1], [r0|r1],
[hn0|hn1], [xn0|xn1]) and of the combined state tile C(s) = [h0(s); h1(s-1)].
Every sigmoid/tanh/DVE op then covers BOTH layers in one [128, fw]
instruction, and layer 1's z/r matmuls contract over the full C (K=128) in
one pass. The batch is further split into independent column streams so the
serial per-step dependency chains overlap on the engines.

Per wavefront step:
  PE : 3 full-width x-side matmuls (start accumulation of zb/rb/nb banks),
       then per stream one cross-layer block matmul per bank (Lr/Lhn/Lz/Lxn,
       K=128 over C, accumulate + stop)
  ACT: R = sigmoid(rb + br)      [128,fw]  (both layers at once)
  DVE: T = (hb + bnh) * R        (scalar_tensor_tensor)
  PE : nb += I @ T               (identity accumulate onto xn)
  ACT: Z = sigmoid(zb + bz);  N = tanh(nb + bni)
  Pool: ZH = Z * C(s-1)          (off critical path)
  DVE: E3 = (Z - 1) * N;  C(s) = ZH - E3   # = z*h + (1-z)*n for both layers
  PE : a few dependency-free dummy matmuls (warm_pe) keep the tensor
       engine's DVFS p-state high between bursts (~10% measured win)
"""

import sys

if "/opt/trn_rl_repo" not in sys.path:
    sys.path.insert(0, "/opt/trn_rl_repo")

import numpy as np
import ml_dtypes

B, T, D, H = 4096, 128, 32, 64
NCORES = 8
BL = B // NCORES  # per-core batch = 512
STREAMS = 2

_CACHE = {}


def _legalize_sync(nc, mybir):
    """Split per-instruction semaphore waits that exceed the ISA wait-slot
    budget into EventSemaphore instructions on the same engine queue."""
    budget = {}  # every instruction type: 1 wait max (walrus adds internal waits)
    ctr = 0
    for f in nc.m.functions:
        for blk in f.blocks:
            out = []
            changed = False
            for inst in blk.instructions:
                si = inst.sync_info
                waits = list(si.on_wait) if (si is not None and si.on_wait) else []
                b = budget.get(type(inst).__name__, 1)
                if len(waits) > b:
                    excess, keep = waits[:-b], waits[-b:]
                    for w in excess:
                        ctr += 1
                        out.append(
                            mybir.InstEventSemaphore(
                                name=f"evw{ctr}_{inst.name}",
                                engine=inst.engine,
                                ins=[],
                                outs=[],
                                sync_info=mybir.SyncInfo(on_wait=[w], on_update=[]),
                            )
                        )
                    si.on_wait = keep
                    changed = True
                out.append(inst)
            if changed:
                try:
                    blk.instructions = out
                except Exception:
                    blk.instructions.clear()
                    blk.instructions.extend(out)
    return ctr


def build_module(t_steps=T, bl=BL, reps=1, streams=STREAMS, blocks=True,
                 idmm="pe", warm_pe=6, pair_mms=False):
    """Build the Bass module (single program, run SPMD on 8 cores).

    reps>1 repeats the whole wavefront (same x, state carried over) for
    slope-timing the device execution under the axon dispatch overhead.
    blocks=True emits each stream's step as one contiguous block (engine
    queues then serve streams phase-shifted instead of head-of-line blocking
    each other); idmm selects PE identity-accumulate vs DVE add for xn + r*hn.
    warm_pe>0 adds that many dependency-free dummy matmuls per step to keep
    the tensor engine's DVFS p-state high between bursts.
    """
    from contextlib import ExitStack

    import concourse.bass as bass
    import concourse.tile as tile
    from concourse import mybir

    f32 = mybir.dt.float32
    bf16 = mybir.dt.bfloat16
    AF = mybir.ActivationFunctionType
    OP = mybir.AluOpType

    fw = bl // streams
    CW = 1032

    nc = bass.Bass()

    x_d = nc.dram_tensor("x", [t_steps, D, bl], bf16, kind="ExternalInput")
    cb_d = nc.dram_tensor("cb", [128, CW], bf16, kind="ExternalInput")
    cf_d = nc.dram_tensor("cf", [128, 8], f32, kind="ExternalInput")
    out_d = nc.dram_tensor("out", [1, bl], f32, kind="ExternalOutput")

    with ExitStack() as ctx:
        tc = ctx.enter_context(tile.TileContext(nc))
        const = ctx.enter_context(tc.tile_pool(name="const", bufs=1))
        work = ctx.enter_context(tc.tile_pool(name="work", bufs=2))
        cpool = ctx.enter_context(tc.tile_pool(name="cpool", bufs=3))
        # with warm_pe a PSUM bank is taken for the dummy matmuls; sacrifice
        # hb's double-buffering (its WAR wait binds latest of the candidates)
        zr_bufs = 1 if warm_pe else 2
        pzb = ctx.enter_context(tc.tile_pool(name="pzb", bufs=zr_bufs, space="PSUM"))
        prb = ctx.enter_context(tc.tile_pool(name="prb", bufs=zr_bufs, space="PSUM"))
        phb = ctx.enter_context(tc.tile_pool(name="phb", bufs=2, space="PSUM"))
        pnb = ctx.enter_context(tc.tile_pool(name="pnb", bufs=2, space="PSUM"))
        pdum = (ctx.enter_context(tc.tile_pool(name="pdum", bufs=1, space="PSUM"))
                if warm_pe else None)

        # ---- constants in SBUF (two packed tiles, two DMAs) ----
        cb = const.tile([128, CW], bf16, tag="cb")
        nc.sync.dma_start(out=cb, in_=cb_d[:])
        cf = const.tile([128, 8], f32, tag="cf")
        nc.sync.dma_start(out=cf, in_=cf_d[:])

        # x-side weights padded to M=128 (right half zero); h-side weights
        # merged across layers into block K=128 matrices (see shard_inputs).
        Xz = cb[0:D, 0:128]
        Xr = cb[0:D, 128:256]
        Xn = cb[0:D, 256:384]
        Lz = cb[0:128, 384:512]
        Lr = cb[0:128, 512:640]
        Lhn = cb[0:128, 640:768]
        Lxn = cb[0:H, 768:896]
        I128 = cb[0:128, 896:1024]
        fcw = cb[H:128, 1024:1025]
        bz = cf[:, 0:1]
        br = cf[:, 1:2]
        bni = cf[:, 2:3]
        bnh = cf[:, 3:4]
        fcb = cf[0:1, 4:5]

        # ACT warm-up: absorbs the sigmoid/tanh table-load and the cf DMA
        # wait into an instruction with spare wait slots.
        warm = work.tile([128, 8], f32, tag="warm")
        nc.scalar.activation(warm, cf, AF.Sigmoid)
        warm_v = work.tile([128, 8], f32, tag="warm_v")
        nc.vector.tensor_copy(warm_v, cf)

        # Preload all of x: 8 chunk tiles written once each.
        CH = max(1, t_steps // 8)
        x_chunks = []
        for c in range(0, t_steps, CH):
            n_t = min(CH, t_steps - c)
            xc = const.tile([D, n_t, bl], bf16, tag=f"xc{c}")
            nc.sync.dma_start(
                out=xc, in_=x_d[c : c + n_t].rearrange("t d b -> d t b")
            )
            x_chunks.append(xc)

        def xs(s, g):
            s = s % t_steps
            return x_chunks[s // CH][:, s % CH, g]

        SG = [slice(sg * fw, (sg + 1) * fw) for sg in range(streams)]
        PL0 = slice(0, H)
        PL1 = slice(H, 128)

        n_steps = t_steps * reps

        C_prev = []
        for sg in range(streams):
            c0 = cpool.tile([128, fw], bf16, tag=f"c{sg}")
            nc.vector.memset(c0, 0.0)
            C_prev.append(c0)

        mm = nc.tensor.matmul

        def emit_xmms(s, zb, rb, nb, l0):
            # x-side matmuls once per step at full batch width (both streams):
            # full-width start=True, per-stream L-matmuls accumulate and stop.
            if not l0:
                return
            xa = xs(s, slice(0, bl))
            mm(rb[:, :], lhsT=Xr, rhs=xa,
               start=True, stop=False, skip_group_check=True)
            mm(zb[:, :], lhsT=Xz, rhs=xa,
               start=True, stop=False, skip_group_check=True)
            mm(nb[:, :], lhsT=Xn, rhs=xa,
               start=True, stop=False, skip_group_check=True)

        def emit_mms(s, sg, zb, rb, hb, nb, l0, l1):
            g, Cp = SG[sg], C_prev[sg]
            mm(rb[:, g], lhsT=Lr, rhs=Cp,
               start=not l0, stop=True, skip_group_check=True)
            mm(hb[:, g], lhsT=Lhn, rhs=Cp,
               start=True, stop=True, skip_group_check=True)
            mm(zb[:, g], lhsT=Lz, rhs=Cp,
               start=not l0, stop=True, skip_group_check=True)
            if l1:
                mm(nb[PL1, g], lhsT=Lxn[:, 64:128], rhs=Cp[PL0, :],
                   start=not l0, stop=idmm != "pe", skip_group_check=True)

        def emit_mms_paired(s, zb, rb, hb, nb, l0, l1):
            # both streams' L-matmuls stage-interleaved so stream 1's chain
            # head (its Lr) sits 2nd in the PE queue, not 5th
            for mat, bank in ((Lr, rb), (Lhn, hb), (Lz, zb)):
                for sg in range(streams):
                    mm(bank[:, SG[sg]], lhsT=mat, rhs=C_prev[sg],
                       start=(not l0) if bank is not hb else True,
                       stop=True, skip_group_check=True)
            if l1:
                for sg in range(streams):
                    mm(nb[PL1, SG[sg]], lhsT=Lxn[:, 64:128],
                       rhs=C_prev[sg][PL0, :],
                       start=not l0, stop=idmm != "pe", skip_group_check=True)

        def emit_sigR(sg, rb, pr):
            R = work.tile([128, fw], bf16, tag=f"R{sg}")
            nc.scalar.activation(R[pr, :], rb[pr, SG[sg]], AF.Sigmoid,
                                 bias=br[pr, :])
            return R

        def emit_sigZ(sg, zb, pr):
            Z = work.tile([128, fw], bf16, tag=f"Z{sg}")
            nc.scalar.activation(Z[pr, :], zb[pr, SG[sg]], AF.Sigmoid,
                                 bias=bz[pr, :])
            Zm1 = work.tile([128, fw], bf16, tag=f"Zm1{sg}")
            nc.vector.tensor_scalar_sub(Zm1[pr, :], Z[pr, :], 1.0)
            return Z, Zm1

        def emit_T(sg, hb, R, pr):
            Tt = work.tile([128, fw], bf16, tag=f"T{sg}")
            nc.vector.scalar_tensor_tensor(
                out=Tt[pr, :], in0=hb[pr, SG[sg]], scalar=bnh[pr, :],
                in1=R[pr, :], op0=OP.add, op1=OP.mult)
            return Tt

        def emit_ZH(sg, Z, pr):
            ZH = work.tile([128, fw], bf16, tag=f"ZH{sg}")
            nc.gpsimd.tensor_mul(ZH[pr, :], Z[pr, :], C_prev[sg][pr, :])
            return ZH

        def emit_n(sg, nb, Tt, pr):
            if idmm == "pe":
                mm(nb[pr, SG[sg]], lhsT=I128[pr, pr], rhs=Tt[pr, :],
                   start=False, stop=True, skip_group_check=True)
                nsrc, nbias = nb[pr, SG[sg]], bni[pr, :]
            else:
                U = work.tile([128, fw], bf16, tag=f"U{sg}")
                nc.vector.scalar_tensor_tensor(
                    out=U[pr, :], in0=nb[pr, SG[sg]], scalar=bni[pr, :],
                    in1=Tt[pr, :], op0=OP.add, op1=OP.add)
                nsrc, nbias = U[pr, :], 0.0
            N = work.tile([128, fw], bf16, tag=f"N{sg}")
            nc.scalar.activation(N[pr, :], nsrc, AF.Tanh, bias=nbias)
            return N

        def emit_tail(s, sg, ZH, Zm1, N, pr):
            E3 = work.tile([128, fw], bf16, tag=f"E3{sg}")
            nc.vector.tensor_mul(E3[pr, :], Zm1[pr, :], N[pr, :])
            Cn = cpool.tile([128, fw], bf16, tag=f"c{sg}")
            nc.vector.tensor_sub(Cn[pr, :], ZH[pr, :], E3[pr, :])
            if s == 0:
                # h1(-1) = 0 for layer 1's first step
                nc.gpsimd.memset(Cn[PL1, :], 0.0)
            return Cn

        for s in range(n_steps + 1):
            l0 = s < n_steps
            l1 = s >= 1
            pr = slice(0 if l0 else H, 128 if l1 else H)

            zb = pzb.tile([128, bl], f32, tag="zb")
            rb = prb.tile([128, bl], f32, tag="rb")
            hb = phb.tile([128, bl], f32, tag="hb")
            nb = pnb.tile([128, bl], f32, tag="nb")

            C_new = [None] * streams
            emit_xmms(s, zb, rb, nb, l0)
            if blocks == "hybrid":
                # all streams' state matmuls first (no idmm head-of-line in
                # the PE queue), elementwise still in per-stream blocks
                for sg in range(streams):
                    emit_mms(s, sg, zb, rb, hb, nb, l0, l1)
                for sg in range(streams):
                    R = emit_sigR(sg, rb, pr)
                    Z, Zm1 = emit_sigZ(sg, zb, pr)
                    Tt = emit_T(sg, hb, R, pr)
                    ZH = emit_ZH(sg, Z, pr)
                    N = emit_n(sg, nb, Tt, pr)
                    C_new[sg] = emit_tail(s, sg, ZH, Zm1, N, pr)
            elif pair_mms:
                emit_mms_paired(s, zb, rb, hb, nb, l0, l1)
                for sg in range(streams):
                    R = emit_sigR(sg, rb, pr)
                    Z, Zm1 = emit_sigZ(sg, zb, pr)
                    Tt = emit_T(sg, hb, R, pr)
                    ZH = emit_ZH(sg, Z, pr)
                    N = emit_n(sg, nb, Tt, pr)
                    C_new[sg] = emit_tail(s, sg, ZH, Zm1, N, pr)
            elif blocks:
                for sg in range(streams):
                    emit_mms(s, sg, zb, rb, hb, nb, l0, l1)
                    R = emit_sigR(sg, rb, pr)
                    Z, Zm1 = emit_sigZ(sg, zb, pr)
                    Tt = emit_T(sg, hb, R, pr)
                    ZH = emit_ZH(sg, Z, pr)
                    N = emit_n(sg, nb, Tt, pr)
                    C_new[sg] = emit_tail(s, sg, ZH, Zm1, N, pr)
            else:
                for sg in range(streams):
                    emit_mms(s, sg, zb, rb, hb, nb, l0, l1)
                Rs = [emit_sigR(sg, rb, pr) for sg in range(streams)]
                ZZ = [emit_sigZ(sg, zb, pr) for sg in range(streams)]
                Ts = [emit_T(sg, hb, Rs[sg], pr) for sg in range(streams)]
                ZHs = [emit_ZH(sg, ZZ[sg][0], pr) for sg in range(streams)]
                Ns = [emit_n(sg, nb, Ts[sg], pr) for sg in range(streams)]
                for sg in range(streams):
                    C_new[sg] = emit_tail(s, sg, ZHs[sg], ZZ[sg][1], Ns[sg], pr)
            if warm_pe:
                dum = pdum.tile([128, bl], f32, tag="dum")
                for _ in range(warm_pe):
                    mm(dum, lhsT=I128, rhs=cb[0:128, 0:512],
                       start=True, stop=True, skip_group_check=True)
            C_prev = C_new

        # final projection: out = fc_w @ h1_final + fc_b  -> [1, bl]
        pfc = pzb.tile([1, bl], f32, tag="zb")
        for sg in range(streams):
            mm(pfc[0:1, SG[sg]], lhsT=fcw, rhs=C_prev[sg][PL1, :],
               start=True, stop=True, skip_group_check=True)
        out_sb = work.tile([1, bl], f32, tag="out")
        nc.scalar.activation(out_sb, pfc, AF.Identity, bias=fcb)
        nc.sync.dma_start(out=out_d[:], in_=out_sb)

    _legalize_sync(nc, mybir)
    return nc


def shard_inputs(inputs, bl=BL, ncores=NCORES, t_steps=T):
    """Host-side prep: transpose/cast/shard full inputs into per-core maps.

    PyTorch/reference GRU gate order in W_ih/W_hh rows is [r | z | n].
    """
    bf = ml_dtypes.bfloat16
    x = np.asarray(inputs["x"], dtype=np.float32)
    xT = np.ascontiguousarray(
        x[: bl * ncores, :t_steps, :].transpose(1, 2, 0)
    ).astype(bf)

    def wT(w, rows):
        w = np.asarray(w, dtype=np.float32)
        return np.ascontiguousarray(w[rows].T).astype(bf)

    R_, Z_, N_ = slice(0, H), slice(H, 2 * H), slice(2 * H, 3 * H)

    CW = 1032
    cb = np.zeros((128, CW), dtype=bf)
    # x-side, zero-padded to M=128
    cb[0:D, 0:64] = wT(inputs["W_ih0"], Z_)
    cb[0:D, 128:192] = wT(inputs["W_ih0"], R_)
    cb[0:D, 256:320] = wT(inputs["W_ih0"], N_)
    # cross-layer block matrices over C = [h0; h1]
    cb[0:H, 384:448] = wT(inputs["W_hh0"], Z_)      # Lz
    cb[0:H, 448:512] = wT(inputs["W_ih1"], Z_)
    cb[H:128, 448:512] = wT(inputs["W_hh1"], Z_)
    cb[0:H, 512:576] = wT(inputs["W_hh0"], R_)      # Lr
    cb[0:H, 576:640] = wT(inputs["W_ih1"], R_)
    cb[H:128, 576:640] = wT(inputs["W_hh1"], R_)
    cb[0:H, 640:704] = wT(inputs["W_hh0"], N_)      # Lhn
    cb[H:128, 704:768] = wT(inputs["W_hh1"], N_)
    cb[0:H, 832:896] = wT(inputs["W_ih1"], N_)      # Lxn right half
    cb[0:128, 896:1024] = np.eye(128, dtype=np.float32).astype(bf)
    cb[H:128, 1024] = np.asarray(inputs["fc_w"], dtype=np.float32).reshape(H).astype(bf)

    cf = np.zeros((128, 8), dtype=np.float32)
    for l, (bi_k, bh_k) in enumerate((("b_ih0", "b_hh0"), ("b_ih1", "b_hh1"))):
        bi = np.asarray(inputs[bi_k], dtype=np.float32)
        bh = np.asarray(inputs[bh_k], dtype=np.float32)
        p = slice(l * H, (l + 1) * H)
        cf[p, 0] = bi[Z_] + bh[Z_]
        cf[p, 1] = bi[R_] + bh[R_]
        cf[p, 2] = bi[N_]
        cf[p, 3] = bh[N_]
    cf[0, 4] = np.asarray(inputs["fc_b"], dtype=np.float32).reshape(())

    shared = {"cb": cb, "cf": cf}

    in_maps = []
    for c in range(ncores):
        m = dict(shared)
        m["x"] = np.ascontiguousarray(xT[:, :, c * bl : (c + 1) * bl])
        in_maps.append(m)
    return in_maps


def kernel(**inputs):
    from concourse import bass_utils

    in_maps = shard_inputs(inputs)
    # Rarely, the first dispatch of a freshly compiled NEFF desyncs the
    # axon worker mesh; a rebuild with a different instruction mix (hence a
    # different NEFF hash) recovers. Retry with progressively simpler builds.
    last_exc = None
    for warm in (6, 5, 0):
        try:
            key = ("nc", warm)
            if key not in _CACHE:
                _CACHE[key] = build_module(warm_pe=warm)
            res = bass_utils.run_bass_kernel_spmd(
                _CACHE[key], in_maps, core_ids=list(range(NCORES)))
            out = np.concatenate([r["out"].reshape(BL) for r in res.results])
            return out.astype(np.float32)
        except Exception as e:  # noqa: BLE001 - retry across transient axon faults
            last_exc = e
    raise last_exc
